# revision 36
# baseline (speedup 1.0000x reference)
"""HMM forward-algorithm kernel for Trainium2 (8 NeuronCores, SPMD data-parallel over batch).

Problem: B=64 sequences, T=1024 steps, S=512 states, V=1024 vocab.
  alpha_0 = emission[obs_0] + prior
  alpha_t[b,j] = emission[obs_t][b,j] + logsumexp_i(alpha_{t-1}[b,i] + trans[i,j])
  out[b] = logsumexp_j(alpha_{T-1}[b,j])

Device algorithm (per core, 8 sequences): run the scan in exp-space,
  phi_t[j,b] = (sum_i expT[i,j] * phi_{t-1}[i,b]) * expE_t[j,b] * (periodic rescale)
with phi kept as a [128, 4*8] bf16 SBUF tile (state chunk c, partition p -> state
s = c*128+p; column c*8+b). The 512x512 exp(trans) matrix lives in SBUF as 16
bf16 [128,128] blocks; each step is 16 PE matmuls (K=128, M=128, N=8).

The per-step schedule is latency-bound on the cycle
  mult(chunk) -> (sem) -> mms reading that chunk -> psum column complete ->
  (sem) -> mult(next) ...
Three psum tiles (ps2/ps3/ps01, one bank each; hardware start_tensor_calc
zeroes the whole bank lazily, so each bank gets exactly ONE start): col 2 and
col 3 complete at mm positions 10/12 with early DVE multiplies; cols {0,1}
share one bank/group and one [128,16] multiply at step end (reading a psum
bank before its accumulation group closes costs ~+125ns of bank contention on
every DVE op -- measured). The next step opens with the chunk-2 readers;
3 filler matmuls keep the matrix fed across the boundary stall; a junk DVE
copy gated on the pos-8 matmul soaks part of the DVE burst-opener cost.

Measured notes (this + prior tuning session, hardware traces):
- LDWEIGHTS [128,128] = 27ns (FWL) regardless of dtype; a matmul pair is
  27ns at N=8 and N=16. The 16 blocks/step = 432ns is this dataflow's floor.
- fp8 plain = same 27ns; fp8 DoubleRow LDWEIGHTS = 127ns; partial-tile
  LDWEIGHTS ([128,32]/[32,128]) correctly preserves the rest of the PE array
  but executes on a 100+165ns slow path. None reduce the weight stream cost.
- A PE pair whose semaphore wait is satisfied at sequencer-decode time runs
  warm (~31ns, microbenched); a pair whose wait genuinely stalls the
  sequencer executes as a ~101+165ns restart regardless of matrix backlog.
  The steady state is self-perpetuating: the slow boundary pair delays col-2
  psum completion, which delays the mult2 semaphore, which re-stalls the next
  boundary (~729ns/step equilibrium, ~= the latency floor given measured
  cross-engine semaphore hops of ~150-190ns).
- Widening moving data to N=16 (don''t-care cols) does not change the pair
  cost but doubles psum write traffic and slows the DVE mults; sequencer-only
  pacing via WRITE costs ~810ns/instr and NOP cycle_cnt is unimplemented.

Every R steps a per-sequence normalizer S1[b] = sum_j phi[j,b] is computed
(2 PE matmuls), applied LAG steps later as a multiply by 1/S1, with log(S1)
accumulated into a per-sequence log-offset C. Final: C[b] + log(sum_j phi[j,b]).

Host side pre-gathers exp(emission_table[obs]) into the packed per-step layout
(pure data movement + exp; the indexing is data-independent of the scan).
"""

import sys

if "/opt/trn_rl_repo" not in sys.path:
    sys.path.insert(0, "/opt/trn_rl_repo")

import numpy as np
import ml_dtypes

import concourse.bass as bass
import concourse.tile as tile
from concourse import bacc
from concourse import mybir

B, T, S, V = 64, 1024, 512, 1024
NCORES = 8
BL = B // NCORES          # 8 sequences per core
NCH = S // 128            # 4 state chunks
PACK = NCH * BL           # 32 packed columns
R_MEAS = 128              # measure normalizer every R steps (bf16 phi
                          # has e^+-88 of range; 128-step drift ~e^43 worst)
LAG = 4                   # apply it this many steps later
DRIFT_COMP = 7.0          # constant log-drift per step, folded into the ES stream
ES_CHUNK = 64             # emission-stream steps per DMA

BF16 = mybir.dt.bfloat16
F32 = mybir.dt.float32
# note: fp8 weights were tried and are NOT faster — trn2 LDWEIGHTS is
# column-rate-limited (27ns per 128-col block regardless of dtype).
# fp8 DoubleRow (K=256/load) was also measured: its LDWEIGHTS takes 127ns,
# and partial-tile LDWEIGHTS ([128,32]/[32,128]) hit a 100+165ns slow path,
# so neither reduces the 27ns/block weight-stream floor.
#
# Measured (microbench): a PE pair whose semaphore wait is ALREADY SATISFIED
# when the sequencer reaches it runs warm (~31ns); the 101+165ns step-boundary
# cost only appears when the sequencer genuinely stalls. So fillers are not
# needed to keep the matrix busy -- only a small amount of sequencer PACING so
# the sequencer arrives at the boundary wait after the gating mult's semaphore
# has fired (~pos-10 matmul + 91ns sem + 73ns mult + 54ns sem ~= 520ns).
import os as _os

N_FILL = int(_os.environ.get("NFILL", "3"))  # boundary-bridging filler pairs
N_FILL_WARMUP = int(_os.environ.get("NFILLWARM", "14"))  # steps 1-6: settle the
# pipeline into the fast steady state deterministically (run-to-run the same
# binary otherwise lands at 729ns/step usually but ~826ns/step occasionally,
# depending on preamble DMA jitter at the first few boundaries)
JUNK_POS = int(_os.environ.get("JUNKPOS", "7"))  # gate junk DVE copy on this mm

# (ci, cj) matmul order per step: ci = input chunk (rhs slice of prev phi),
# cj = output column (psum). From offline steady-state search at RT~=405ns.
# B-block first: 8 mms reading chunks {2,3} (gated by multB of the previous
# step), then the A-block reading chunks {0,1} (gated by multA). Within each
# block, the mms feeding columns {2,3} come first so multB issues after only
# 4 A-block mms; multA issues at the end. The A-gated block starts right as
# the B-block drains, so the PE queue never empties and refills stay hidden.
MM_ORDER_INTERLEAVED = [
    (2, 2), (2, 3), (2, 0), (2, 1),
    (3, 2), (3, 3), (3, 0), (3, 1),
    (0, 2), (1, 2), (0, 3), (1, 3),
    (0, 0), (1, 0), (0, 1), (1, 1),
]
MM_ORDER_COLGROUPED = [
    (2, 2), (3, 2), (0, 2), (1, 2),
    (2, 3), (3, 3), (0, 3), (1, 3),
    (2, 0), (3, 0), (0, 0), (1, 0),
    (2, 1), (3, 1), (0, 1), (1, 1),
]
# col2's writers at positions 1,2,5,6: its chunk-0/1 readers sit at 5-6 so
# the previous step's final mult has ~100ns more headroom than col-grouped,
# while col2 still completes 4 positions earlier than the interleaved order.
MM_ORDER_EARLY2 = [
    (2, 2), (3, 2), (2, 3), (3, 3),
    (0, 2), (1, 2), (2, 0), (3, 0),
    (0, 3), (1, 3), (2, 1), (3, 1),
    (0, 0), (1, 0), (0, 1), (1, 1),
]
_ORDERS = {"COLGROUPED": MM_ORDER_COLGROUPED, "EARLY2": MM_ORDER_EARLY2}
MM_ORDER = _ORDERS.get(_os.environ.get("MMORDER", ""), MM_ORDER_INTERLEAVED)
# mm_pos -> DVE mults to emit there, derived from MM_ORDER: a column's mult
# fires at its 4th writer; cols 0+1 share a psum bank so their combined mult
# fires when BOTH have finished (one [128,16] op, after the bank's group
# closes -- reading a psum bank mid-group costs ~125ns contention per DVE op).
def _mult_positions(order):
    last = {}
    for p, (ci, cj) in enumerate(order):
        last[cj] = p
    out = {last[2]: [2], last[3]: [3], max(last[0], last[1]): [0, 1]}
    return out

MULT_COLS = _mult_positions(MM_ORDER)

# Moving-data width per matmul. The PE matrix pair costs ~27ns at N=8 and
# ~31ns at N=16 (still FWL-bound; the sequencer issues pairs at ~31ns). With
# N=8 the matrix (27) outruns the sequencer (31) and drains to empty at every
# step boundary, forcing a ~266ns pipeline-restart pair each step. Widening
# the moving data to N=16 (8 real sequences + 8 don't-care columns) matches
# the matrix rate to the sequencer rate, so the matrix never drains and every
# boundary pair stays warm. The don't-care psum columns are never read.
MOVN = int(_os.environ.get("MOVN", "8"))
# per-column DVE multiplies; completion order (by MM_ORDER): col2 @ mm pos 10,
# col3 @ 12, col0 @ 14, col1 @ 16. The next step opens with the chunk-2
# readers, so the exposed round trip is sem+mult2 only.
GROUPS = ((2,), (3,), (0,), (1,))


def build_tile_body(tc, w_ap, phi0_ap, es_ap, c0_ap, ones128_ap, sel_ap, ones1_ap, out_ap, n_steps):
    """Emit the full scan. n_steps = number of recurrence steps (T-1)."""
    nc = tc.nc
    import contextlib

    ctx = contextlib.ExitStack()
    with ctx:
        wpool = ctx.enter_context(tc.tile_pool(name="w", bufs=1))
        espool = ctx.enter_context(tc.tile_pool(name="es", bufs=3))
        phipool = ctx.enter_context(tc.tile_pool(name="phi", bufs=3))
        pspool = ctx.enter_context(tc.tile_pool(name="ps", bufs=2, space="PSUM"))
        pssmall = ctx.enter_context(tc.tile_pool(name="pss", bufs=1, space="PSUM"))
        nrmpool = ctx.enter_context(tc.tile_pool(name="nrm", bufs=4))
        accpool = ctx.enter_context(tc.tile_pool(name="acc", bufs=1))

        wt = wpool.tile([128, NCH * NCH * 128], BF16)
        nc.sync.dma_start(wt[:], w_ap[:])

        # phi tiles carry 8 extra don't-care columns so chunk-3's N=16 moving
        # read [128, 24:40] stays in bounds; those columns are memset once per
        # physical buffer (they are never written afterwards)
        phi = phipool.tile([128, PACK + MOVN - BL], BF16, tag="phi")
        nc.sync.dma_start(phi[:, 0:PACK], phi0_ap[:])
        if MOVN > BL:
            nc.vector.memset(phi[:, PACK:], 0.0)

        cacc = accpool.tile([1, BL], F32)
        nc.sync.dma_start(cacc[:], c0_ap[:])

        ones128_t = accpool.tile([128, 1], BF16, tag="ones128")
        nc.sync.dma_start(ones128_t[:], ones128_ap[:])
        sel_t = accpool.tile([PACK, BL], BF16, tag="sel")
        nc.sync.dma_start(sel_t[:], sel_ap[:])
        ones1_t = accpool.tile([1, 128], BF16, tag="ones1")
        nc.sync.dma_start(ones1_t[:], ones1_ap[:])
        # full [128,128] ones weight: lets col_sums use a warm full-size
        # LDWEIGHTS instead of the 100+165ns odd-size slow path
        onesw = accpool.tile([128, 128], BF16, tag="onesw")
        nc.vector.memset(onesw[:], 1.0)

        pending = {}  # apply_step -> (rb_tile, lns_tile)
        prev_mm = None
        prev_tt = None

        esc = None
        esc_len = 0
        esc_start = 0

        def col_sums(src_phi, out_dtype):
            """[1, BL] per-sequence sums of src_phi: ONE full-size ones-weight
            matmul (warm ~27ns pair; every psum partition holds the column
            sums) + ACT copy of row 0 + tiny GpSimd adds for the 4-chunk
            combine. Replaces two odd-size weight loads that each paid the
            100+165ns PE slow path and perturbed the steady state."""
            nonlocal prev_mm
            rs = pssmall.tile([128, 48], F32, tag="rs", name="rs")
            m = nc.tensor.matmul(rs[:, 0:PACK], onesw[:], src_phi[:, 0:PACK],
                                 start=True, stop=True)
            if prev_mm is not None:
                tile.add_dep_helper(m.ins, prev_mm.ins, sync=False, reason="pe order")
            prev_mm = m
            ppsb = nrmpool.tile([1, PACK], F32, tag="ppsb")
            nc.scalar.copy(ppsb[:], rs[0:1, 0:PACK])
            comb = nrmpool.tile([1, 2 * BL], F32, tag="comb")
            nc.gpsimd.tensor_tensor(comb[0:1, 0:BL], ppsb[0:1, 0:BL],
                                    ppsb[0:1, BL:2 * BL], mybir.AluOpType.add)
            nc.gpsimd.tensor_tensor(comb[0:1, BL:2 * BL], ppsb[0:1, 2 * BL:3 * BL],
                                    ppsb[0:1, 3 * BL:4 * BL], mybir.AluOpType.add)
            s1b = nrmpool.tile([1, BL], out_dtype, tag="s1b")
            nc.gpsimd.tensor_tensor(s1b[:], comb[0:1, 0:BL], comb[0:1, BL:2 * BL],
                                    mybir.AluOpType.add)
            return s1b

        def measure(src_phi, t):
            """rb = bf16(1/S1), lnrb = ln(rb) exactly as applied."""
            s1b = col_sums(src_phi, F32)
            # reciprocal+cast on DVE (tiny [1,8] ops, once per R_MEAS steps);
            # ACT only runs Ln/Copy so its function table is never reloaded
            # (adding Exp here costs a 1283ns ACT_TABLE_LOAD per rescale)
            rbf = nrmpool.tile([1, BL], F32, tag="rbf")
            nc.vector.reciprocal(rbf[:], s1b[:])
            rb = nrmpool.tile([1, BL], BF16, tag="rb")
            nc.vector.tensor_copy(rb[:], rbf[:])
            lnrb = nrmpool.tile([1, BL], F32, tag="lnrb")
            nc.scalar.activation(lnrb[:], rb[:], mybir.ActivationFunctionType.Ln)
            return rb, lnrb

        def apply_rescale(dst_phi, rb, lnrb):
            nonlocal prev_mm
            rsb = pssmall.tile([128, 48], F32, tag="rs", name="rsb")
            rbB = rsb[:, 16 : 16 + PACK]
            m = nc.tensor.matmul(
                rbB,
                ones1_t[:],
                rb[:, None, :].to_broadcast((1, NCH, BL)),
                start=True,
                stop=True,
            )
            if prev_mm is not None:
                tile.add_dep_helper(m.ins, prev_mm.ins, sync=False, reason="pe order")
            prev_mm = m
            nc.vector.tensor_tensor(
                dst_phi[:, 0:PACK], dst_phi[:, 0:PACK], rbB, mybir.AluOpType.mult
            )
            # cacc update on GpSimd: both operands SBUF, keeps DVE FIFO clear
            nc.gpsimd.tensor_sub(cacc[:], cacc[:], lnrb[:])

        esc_next = None

        def dma_es_chunk(start):
            ln = min(ES_CHUNK, n_steps - start)
            tl = espool.tile([128, ES_CHUNK * PACK], BF16, tag="esc", name="esc")
            nc.sync.dma_start(
                tl[:, : ln * PACK],
                es_ap[:, start * PACK : (start + ln) * PACK],
            )
            return tl, ln

        for t in range(1, n_steps + 1):
            # emission stream chunk; the NEXT chunk's DMA is prefetched a full
            # chunk ahead so no step ever waits on the ~900ns DMA semaphore
            idx = t - 1
            if esc is None:
                esc_start = 0
                esc, esc_len = dma_es_chunk(0)
            elif idx >= esc_start + esc_len:
                esc_start = idx
                esc, esc_len = esc_next
                esc_next = None
            if idx == esc_start + 1 and esc_start + esc_len < n_steps:
                esc_next = dma_es_chunk(esc_start + esc_len)
            off = idx - esc_start

            newphi = phipool.tile([128, PACK + MOVN - BL], BF16, tag="phi")
            if t <= 2 and MOVN > BL:
                # zero the don't-care columns of this physical buffer once
                # (3 buffers total: phi0's plus t=1, t=2; never written after)
                ms = nc.vector.memset(newphi[:, PACK:], 0.0)
                if prev_tt is not None:
                    tile.add_dep_helper(ms.ins, prev_tt.ins, sync=False, reason="dve order")
                prev_tt = ms
            # three psum tiles (separate banks): start_tensor_calc zeroes the
            # whole bank lazily, so each bank gets exactly ONE start (its first
            # writer); cols 0+1 share a tile with a single 8-matmul group.
            # Column cj's real half = [coff : coff+BL]; [coff+BL : coff+MOVN]
            # holds the don't-care half of the N=16 matmuls.
            ps2 = pspool.tile([128, MOVN], F32, tag="ps2", name="ps2")
            ps3 = pspool.tile([128, MOVN], F32, tag="ps3", name="ps3")
            ps01 = pspool.tile([128, 2 * MOVN], F32, tag="ps01", name="ps01")
            tmap = {2: (0, ps2, 0), 3: (1, ps3, 0), 0: (2, ps01, 0), 1: (2, ps01, MOVN)}
            tsize = {0: 4, 1: 4, 2: 8}

            tseen = [0, 0, 0]
            colseen = [0] * NCH
            for mm_pos, (ci, cj) in enumerate(MM_ORDER):
                ti, pst, coff = tmap[cj]
                start = tseen[ti] == 0
                tseen[ti] += 1
                stop = tseen[ti] == tsize[ti]
                colseen[cj] += 1
                m = nc.tensor.matmul(
                    pst[:, coff : coff + MOVN],
                    wt[:, (ci * NCH + cj) * 128 : (ci * NCH + cj + 1) * 128],
                    phi[:, ci * BL : ci * BL + MOVN],
                    start=start,
                    stop=stop,
                )
                if prev_mm is not None:
                    tile.add_dep_helper(m.ins, prev_mm.ins, sync=False, reason="pe order")
                prev_mm = m

                if mm_pos == JUNK_POS:
                    # junk DVE copy, gated on this mid-step matmul so it lands
                    # right before mult2's psum semaphore arrives: it absorbs
                    # the ~93ns DVE burst-opener cost that would otherwise sit
                    # on the critical mult2 -> next-step-boundary path
                    dvw = nrmpool.tile([1, BL], BF16, tag="dvw", bufs=2, name="dvw")
                    jt = nc.vector.tensor_copy(dvw[:], ones1_t[0:1, 0:BL])
                    tile.add_dep_helper(jt.ins, m.ins, sync=True, reason="junk pace")
                    if prev_tt is not None:
                        tile.add_dep_helper(jt.ins, prev_tt.ins, sync=False, reason="dve order")
                    prev_tt = jt

                mcs = MULT_COLS.get(mm_pos, ())
                if mcs == (0, 1) or mcs == [0, 1]:
                    # cols 0+1 finish together (shared bank): one [128,16] mult
                    es_off = off * PACK
                    tt = nc.vector.tensor_tensor(
                        newphi[:, 0 : 2 * BL],
                        ps01[:, 0 : 2 * BL],
                        esc[:, es_off : es_off + 2 * BL],
                        mybir.AluOpType.mult,
                    )
                    if prev_tt is not None:
                        tile.add_dep_helper(tt.ins, prev_tt.ins, sync=False, reason="dve order")
                    prev_tt = tt
                else:
                    for mc in mcs:
                        # column mc's psum bank group closed: multiply its real
                        # half by the emission slice right away (early mults
                        # gate the next step's matmuls reading this chunk)
                        mti, mpst, mcoff = tmap[mc]
                        es_off = off * PACK + mc * BL
                        tt = nc.vector.tensor_tensor(
                            newphi[:, mc * BL : mc * BL + BL],
                            mpst[:, mcoff : mcoff + BL],
                            esc[:, es_off : es_off + BL],
                            mybir.AluOpType.mult,
                        )
                        if prev_tt is not None:
                            tile.add_dep_helper(tt.ins, prev_tt.ins, sync=False, reason="dve order")
                        prev_tt = tt

            # filler chain: keeps the PE queue non-empty through the
            # multB -> semaphore window so the next step's first
            # LDWEIGHTS+MATMUL run warm (~54ns) instead of paying a cold
            # restart (~295ns measured: 105ns LDW + 189ns refill matmul)
            # one ACCUMULATION GROUP (start only on the first mm) so the
            # fillers chain back-to-back at ~27ns like the real psum groups;
            # bufs=2 puts the WAR partner a full step in the past
            # filler chain: keeps the PE queue non-empty across the boundary
            # stall; one accumulation group so they chain at ~27ns
            nf = N_FILL_WARMUP if t <= 6 else N_FILL
            fill = pssmall.tile([128, MOVN], F32, tag="fill", name="fill", bufs=1)
            for _f in range(nf):
                fm = nc.tensor.matmul(
                    fill[:], wt[:, 0:128], phi[:, 0:MOVN],
                    start=(_f == 0), stop=(_f == nf - 1),
                )
                tile.add_dep_helper(fm.ins, prev_mm.ins, sync=False, reason="pe order")
                prev_mm = fm

            if t in pending:
                rb, lns = pending.pop(t)
                apply_rescale(newphi, rb, lns)

            if t % R_MEAS == 0 and t < n_steps:
                # measure the PREVIOUS step's phi: same exactness (lnrb matches
                # the rb actually applied), but the col_sums matmuls no longer
                # wait on this step's DVE multiply -- they fill the boundary gap
                pending[t + LAG] = measure(phi, t)

            phi = newphi

        # flush remaining rescales into the final phi
        for t in sorted(pending):
            rb, lns = pending.pop(t)
            apply_rescale(phi, rb, lns)

        # final logsumexp: out = C + ln(sum_j phi)
        s1f = col_sums(phi, F32)
        lns = nrmpool.tile([1, BL], F32, tag="lns")
        nc.scalar.activation(lns[:], s1f[:], mybir.ActivationFunctionType.Ln)
        outt = accpool.tile([1, BL], F32, tag="outt")
        nc.vector.tensor_add(outt[:], cacc[:], lns[:])
        nc.sync.dma_start(out_ap[:], outt[:])


def build_program(n_steps, compile=True):
    nc = bacc.Bacc(None)
    w = nc.dram_tensor("w", [128, NCH * NCH * 128], BF16, kind="ExternalInput")
    phi0 = nc.dram_tensor("phi0", [128, PACK], BF16, kind="ExternalInput")
    es = nc.dram_tensor("es", [128, n_steps * PACK], BF16, kind="ExternalInput")
    c0 = nc.dram_tensor("c0", [1, BL], F32, kind="ExternalInput")
    ones128 = nc.dram_tensor("ones128", [128, 1], BF16, kind="ExternalInput")
    sel = nc.dram_tensor("sel", [PACK, BL], BF16, kind="ExternalInput")
    ones1 = nc.dram_tensor("ones1", [1, 128], BF16, kind="ExternalInput")
    out = nc.dram_tensor("out", [1, BL], F32, kind="ExternalOutput")
    with tile.TileContext(nc) as tc:
        build_tile_body(tc, w, phi0, es, c0, ones128, sel, ones1, out, n_steps)
    if compile:
        nc.compile()
    return nc


def host_prepare(observations, emission_table, transitions, prior, n_steps=None):
    """Build per-core input dicts. n_steps defaults to T-1."""
    obs = np.asarray(observations)
    table = np.asarray(emission_table, dtype=np.float32)
    trans = np.asarray(transitions, dtype=np.float32)
    prior = np.asarray(prior, dtype=np.float32)
    Tn = obs.shape[1]
    if n_steps is None:
        n_steps = Tn - 1

    eT = np.exp(trans)
    w = np.empty((128, NCH * NCH * 128), dtype=ml_dtypes.bfloat16)
    for ci in range(NCH):
        for cj in range(NCH):
            w[:, (ci * NCH + cj) * 128 : (ci * NCH + cj + 1) * 128] = eT[
                ci * 128 : (ci + 1) * 128, cj * 128 : (cj + 1) * 128
            ]

    in_maps = []
    for c in range(NCORES):
        bsl = slice(c * BL, (c + 1) * BL)
        E0 = table[obs[bsl, 0]] + prior  # [BL, S]
        c0 = E0.max(axis=1)  # [BL]
        phi0 = np.exp(E0 - c0[:, None])  # [BL, S]
        # pack [BL, S] -> [128, (c b)]
        phi0p = (
            phi0.reshape(BL, NCH, 128).transpose(2, 1, 0).reshape(128, PACK)
        ).astype(ml_dtypes.bfloat16)

        # emission stream for steps 1..n_steps: [128, n_steps*PACK]
        rows = table[obs[bsl, 1 : 1 + n_steps]]  # [BL, n_steps, S]
        ex = np.exp(rows - DRIFT_COMP).reshape(BL, n_steps, NCH, 128)
        esp = (
            ex.transpose(3, 1, 2, 0).reshape(128, n_steps * PACK)
        ).astype(ml_dtypes.bfloat16)

        sel = np.zeros((PACK, BL), dtype=ml_dtypes.bfloat16)
        for cc in range(NCH):
            for b in range(BL):
                sel[cc * BL + b, b] = 1
        in_maps.append(
            {
                "w": w,
                "phi0": phi0p,
                "es": esp,
                "c0": (c0 + DRIFT_COMP * n_steps).reshape(1, BL).astype(np.float32),
                "ones128": np.ones((128, 1), dtype=ml_dtypes.bfloat16),
                "sel": sel,
                "ones1": np.ones((1, 128), dtype=ml_dtypes.bfloat16),
            }
        )
    return in_maps


_CACHE = {}


def _get_program(n_steps):
    if n_steps not in _CACHE:
        _CACHE[n_steps] = build_program(n_steps)
    return _CACHE[n_steps]


def kernel(observations, emission_table, transitions, prior):
    from concourse.bass_utils import run_bass_kernel_spmd

    nc = _get_program(T - 1)
    in_maps = host_prepare(observations, emission_table, transitions, prior)
    res = run_bass_kernel_spmd(nc, in_maps, core_ids=list(range(NCORES)))
    out = np.concatenate([r["out"].reshape(BL) for r in res.results])
    return out.astype(np.float32)

